# revision 22
# baseline (speedup 1.0000x reference)
"""SAGAN-style attention block on 8 TRN2 NeuronCores, data-parallel over batch.

Reference computation (per image, x: [64, 64, 512]):
    theta = x @ W_theta                     [4096, 64]
    phi   = maxpool2x2(x @ W_phi)           [1024, 64]
    g     = maxpool2x2(x @ W_g)             [1024, 256]
    beta  = softmax(theta @ phi.T, axis=-1) [4096, 1024]
    o     = (beta @ g) @ W_o                [4096, 512]
    out   = gamma * o + x
Sharding: batch 16 -> 2 images per core. No collectives.

Layout strategy (v3):
  - x is transposed and cast to bf16 on the HOST (xT [C, S]); no on-chip
    transpose phase. Residual x stays f32 via a separate input.
  - All matmuls run in bf16 (fast LDWEIGHTS, f32 PSUM accumulation).
  - Projections feature-major: thph [128, S] holds theta rows 0:64 and the
    phi pre-pool scratch rows 64:128 (one PSUM evacuation per q-tile);
    phiT [64, SK] (maxpool in the free dim), gT [256, SK] -> PE-transposed
    to key-major g_aug[kc] [128, 256].
  - scoresT [k, q] per (qt, kc) written pairwise into wide [128, 1024] PSUM
    tiles; ScalarE applies Exp once per pair on the PSUM->SBUF evacuation
    (no max subtraction: |scores| < 60, safe in f32).
  - Softmax denominators: pairwise-sum tree of the 4 wide exp tiles
    (DVE + GpSimd) -> es [128, 512]; PE-transpose es to q-major; DVE
    reduce + reciprocal gives rec [128q, 4]. No per-kc ones-matmuls.
  - attnV vc-outer: tmpT[v, q] accumulated over kc per vc so tT[0] is
    evacuated (ScalarE, bf16) while vc=1 still accumulates.
  - Output projection for q-tile N is emitted after the scores of q-tile
    N+1 (software pipelining) so the PE never waits on the tT evacuation.
  - out = (tmpT.T @ (gamma*W_o)) * rec + x fused on DVE, stored per-qs.
"""
import sys
import numpy as np

sys.path.insert(0, "/opt/trn_rl_repo")

from contextlib import ExitStack

import concourse.bass as bass
import concourse.tile as tile
from concourse import bacc, mybir
from concourse.bass_utils import run_bass_kernel_spmd

N_CORES = 8
IMG = 2            # images per core
H = W = 64
S = H * W          # 4096 queries per image
SK = S // 4        # 1024 keys after 2x2 maxpool
C = 512
D = C // 8         # 64
V = C // 2         # 256
QT = 512           # query tile
NQT = S // QT      # 8

F32 = mybir.dt.float32
BF16 = mybir.dt.bfloat16

_CACHED_NC = None


def _build():
    nc = bacc.Bacc("TRN2", target_bir_lowering=False, debug=False,
                   num_devices=N_CORES)
    xt_d = nc.dram_tensor("xt", [IMG, C, S], BF16, kind="ExternalInput").ap()
    xr_d = nc.dram_tensor("xr", [IMG, S, C], BF16, kind="ExternalInput").ap()
    wtp_d = nc.dram_tensor("wtp", [128, 4 * 128], BF16, kind="ExternalInput").ap()
    wg_d = nc.dram_tensor("wg", [128, 4 * V], BF16, kind="ExternalInput").ap()
    wo_d = nc.dram_tensor("wo", [128, 2 * C], BF16, kind="ExternalInput").ap()
    id_d = nc.dram_tensor("ident", [128, 128], BF16, kind="ExternalInput").ap()
    out_d = nc.dram_tensor("out", [IMG, S, C], BF16, kind="ExternalOutput").ap()

    AO = mybir.AluOpType
    dma_engines = [nc.sync, nc.scalar, nc.sync, nc.scalar]

    with tile.TileContext(nc) as tc, ExitStack() as ctx:
        # SBUF pools
        const_p = ctx.enter_context(tc.tile_pool(name="const", bufs=1))
        xt_p = ctx.enter_context(tc.tile_pool(name="xt", bufs=1))
        th_p = ctx.enter_context(tc.tile_pool(name="th", bufs=1))
        ga_p = ctx.enter_context(tc.tile_pool(name="ga", bufs=1))
        scr_p = ctx.enter_context(tc.tile_pool(name="scr", bufs=2))
        exp_p = ctx.enter_context(tc.tile_pool(name="exp", bufs=8))
        esw_p = ctx.enter_context(tc.tile_pool(name="esw", bufs=4))
        es_p = ctx.enter_context(tc.tile_pool(name="es", bufs=2))
        rc_p = ctx.enter_context(tc.tile_pool(name="rc", bufs=4))
        tt_p = ctx.enter_context(tc.tile_pool(name="tt", bufs=4))
        xr_p = ctx.enter_context(tc.tile_pool(name="xr", bufs=3))
        o_p = ctx.enter_context(tc.tile_pool(name="o", bufs=3))
        # PSUM pools (8 banks of 2KB/partition total)
        psA = ctx.enter_context(tc.tile_pool(name="psA", bufs=2, space="PSUM"))
        psTV = ctx.enter_context(tc.tile_pool(name="psTV", bufs=2, space="PSUM"))
        psO = ctx.enter_context(tc.tile_pool(name="psO", bufs=2, space="PSUM"))

        # xT wave layout: two 512-col half-waves first (first projections
        # start after 0.5MB), then three 1024-col waves.
        W_OFF = [0, 512, 1024, 2048, 3072]
        W_LEN = [512, 512, 1024, 1024, 1024]
        # maps qt -> (wave index, column offset within the wave)
        QT_WAVE = [(0, 0), (1, 0), (2, 0), (2, 512),
                   (3, 0), (3, 512), (4, 0), (4, 512)]

        def make_xt_tiles():
            return [[xt_p.tile([128, W_LEN[h]], BF16, tag=f"xT{cc}h{h}",
                               name=f"xT{cc}h{h}") for h in range(5)]
                    for cc in range(4)]

        def load_xt_wave(img, tiles, h):
            for cc in range(4):
                dma_engines[(h * 4 + cc) % 2].dma_start(
                    tiles[cc][h][:],
                    xt_d[img, cc * 128:(cc + 1) * 128,
                         W_OFF[h]:W_OFF[h] + W_LEN[h]])

        # img0 first half-wave goes out before anything else (DMA-bound start)
        xTh0 = make_xt_tiles()
        load_xt_wave(0, xTh0, 0)

        # --- constants / weights (already packed + bf16 on host) ---
        wtp = const_p.tile([128, 4 * 128], BF16, tag="wtp", name="wtp")
        nc.sync.dma_start(wtp[:], wtp_d[:])
        ident_b = const_p.tile([128, 128], BF16, tag="ident", name="ident_b")
        nc.sync.dma_start(ident_b[:], id_d[:])
        wg = const_p.tile([128, 4 * V], BF16, tag="wg", name="wg")
        nc.sync.dma_start(wg[:], wg_d[:])
        wo = const_p.tile([128, 2 * C], BF16, tag="wo", name="wo")
        nc.sync.dma_start(wo[:], wo_d[:])

        for img in range(IMG):
            # ---------- load xT (host-transposed, bf16, quarter waves) ----------
            if img == 0:
                xTh = xTh0
                for h in range(1, 5):
                    load_xt_wave(0, xTh, h)
            else:
                xTh = make_xt_tiles()
                for h in range(5):
                    load_xt_wave(img, xTh, h)

            # ---------- projections (feature-major) ----------
            # thph rows 0:64 = thetaT [64, S]; rows 64:128 = phi pre-pool scratch
            thph = th_p.tile([128, S], BF16, tag="thph", name="thph")
            thetaT = thph[0:64, :]
            phiT = th_p.tile([64, SK], BF16, tag="phiT", name="phiT")
            gTp = [th_p.tile([128, SK], BF16, tag=f"gTp{vc}", name=f"gTp{vc}")
                   for vc in range(2)]

            for qt in range(NQT):
                qsl = slice(qt * QT, (qt + 1) * QT)
                hh, hoff = QT_WAVE[qt]
                hsl = slice(hoff, hoff + QT)
                tp_ps = psA.tile([128, 2 * QT], F32, tag="mm", name="mm")
                for cc in range(4):
                    nc.tensor.matmul(tp_ps[:, 0:QT],
                                     wtp[:, cc * 128:(cc + 1) * 128],
                                     xTh[cc][hh][:, hsl], start=(cc == 0), stop=(cc == 3))
                # one evacuation for theta+phi
                nc.vector.tensor_copy(thph[:, qsl], tp_ps[:, 0:QT])
                # phi: maxpool 2x2 in the free dim:
                # QT=512 covers 8 rows of w=64 -> [p, a=4, b=2, c=32, d=2]
                pv = thph[64:128, qsl].rearrange(
                    "p (a b c d) -> p a b c d", b=2, c=32, d=2)
                m1 = scr_p.tile([64, 256], BF16, tag="m1", name="m1")
                ma = m1[:, :128].rearrange("p (a c) -> p a c", a=4)
                mb = m1[:, 128:].rearrange("p (a c) -> p a c", a=4)
                po = phiT[:, qt * 128:(qt + 1) * 128].rearrange(
                    "p (a c) -> p a c", a=4)
                nc.vector.tensor_tensor(ma, pv[:, :, 0, :, 0], pv[:, :, 0, :, 1],
                                        AO.max)
                nc.vector.tensor_tensor(mb, pv[:, :, 1, :, 0], pv[:, :, 1, :, 1],
                                        AO.max)
                nc.vector.tensor_tensor(po, ma, mb, AO.max)

                for vc in range(2):
                    g_ps = psO.tile([128, QT], F32, tag="mm", name="mm")
                    for cc in range(4):
                        nc.tensor.matmul(
                            g_ps[:], wg[:, cc * V + vc * 128: cc * V + (vc + 1) * 128],
                            xTh[cc][hh][:, hsl], start=(cc == 0), stop=(cc == 3))
                    g_sb = scr_p.tile([128, QT], BF16, tag="g_sb", name="g_sb")
                    nc.scalar.copy(g_sb[:], g_ps[:])
                    gv = g_sb.rearrange("p (a b c d) -> p a b c d", b=2, c=32, d=2)
                    m2 = scr_p.tile([128, 256], BF16, tag="m2", name="m2")
                    ga = m2[:, :128].rearrange("p (a c) -> p a c", a=4)
                    gb = m2[:, 128:].rearrange("p (a c) -> p a c", a=4)
                    go = gTp[vc][:, qt * 128:(qt + 1) * 128].rearrange(
                        "p (a c) -> p a c", a=4)
                    nc.vector.tensor_tensor(ga, gv[:, :, 0, :, 0], gv[:, :, 0, :, 1],
                                            AO.max)
                    nc.vector.tensor_tensor(gb, gv[:, :, 1, :, 0], gv[:, :, 1, :, 1],
                                            AO.max)
                    nc.vector.tensor_tensor(go, ga, gb, AO.max)

            # g -> key-major: g_aug[kc] = [128k, 256v] (bf16 PE transpose)
            g_aug = []
            for kc in range(8):
                ga_t = ga_p.tile([128, V], BF16, tag=f"gaug{kc}", name=f"gaug{kc}")
                tp = psO.tile([128, 512], BF16, tag="mm", name="mm")
                for vc in range(2):
                    nc.tensor.transpose(tp[:, vc * 128:(vc + 1) * 128],
                                        gTp[vc][:, kc * 128:(kc + 1) * 128],
                                        ident_b[:])
                nc.vector.tensor_copy(ga_t[:], tp[:, 0:V])
                g_aug.append(ga_t)

            # ---------- attention, software-pipelined over q-tiles ----------
            prev = None  # (tT, rec, xr, qt) of the previous q-tile

            def emit_outproj(state):
                tT, rec, xr, qt = state
                for qs in range(4):
                    ssl = slice(qs * 128, (qs + 1) * 128)
                    o_ps = psO.tile([128, C], F32, tag="mm", name="mm")
                    for vc in range(2):
                        nc.tensor.matmul(o_ps[:], tT[vc][:, ssl],
                                         wo[:, vc * C:(vc + 1) * C],
                                         start=(vc == 0), stop=(vc == 1))
                    ot = o_p.tile([128, C], BF16, tag="ot", name="ot")
                    # out = (o * (1/sum)) + x   (gamma pre-folded into W_o)
                    nc.vector.scalar_tensor_tensor(
                        ot[:], o_ps[:], rec[:, qs:qs + 1],
                        xr[:, qs * C:(qs + 1) * C], AO.mult, AO.add)
                    q0 = qt * QT + qs * 128
                    nc.sync.dma_start(out_d[img, q0:q0 + 128, :], ot[:])

            for qt in range(NQT):
                qsl = slice(qt * QT, (qt + 1) * QT)
                # residual x for this q-tile (consumed next iteration)
                xr = xr_p.tile([128, 4 * C], BF16, tag="xr", name="xr")
                nc.sync.dma_start(
                    xr.rearrange("p (a c) -> p a c", a=4),
                    xr_d[img, qt * QT:(qt + 1) * QT, :].rearrange(
                        "(a p) c -> p a c", p=128))

                # scoresT [k, q] pairwise into wide PSUM; one Exp per pair
                ex = []
                for kp in range(4):
                    sc_ps = psA.tile([128, 2 * QT], F32, tag="mm", name="mm")
                    for h in range(2):
                        kc = 2 * kp + h
                        nc.tensor.matmul(sc_ps[:, h * QT:(h + 1) * QT],
                                         phiT[:, kc * 128:(kc + 1) * 128],
                                         thetaT[:, qsl], start=True, stop=True)
                    e_t = exp_p.tile([128, 2 * QT], BF16, tag="exp", name="exp")
                    nc.scalar.activation(e_t[:], sc_ps[:],
                                         mybir.ActivationFunctionType.Exp)
                    ex.append(e_t)

                # output projection of the PREVIOUS q-tile (pipelined):
                # PE fills the wait on exp with useful work, DVE runs the
                # STTs while the scores stream
                if prev is not None:
                    emit_outproj(prev)

                # denominator pre-sum tree (DVE)
                w01 = esw_p.tile([128, 2 * QT], BF16, tag="esw", name="esw")
                w23 = esw_p.tile([128, 2 * QT], BF16, tag="esw", name="esw")
                nc.vector.tensor_tensor(w01[:], ex[0][:], ex[1][:], AO.add)
                nc.vector.tensor_tensor(w23[:], ex[2][:], ex[3][:], AO.add)
                wsum = esw_p.tile([128, 2 * QT], BF16, tag="esw", name="esw")
                nc.vector.tensor_tensor(wsum[:], w01[:], w23[:], AO.add)
                es = es_p.tile([128, QT], BF16, tag="es", name="es")
                nc.vector.tensor_tensor(es[:], wsum[:, 0:QT], wsum[:, QT:2 * QT],
                                        AO.add)

                # attnV vc-outer: tT[vc] evacuates while the other vc runs
                tT = [tt_p.tile([128, QT], BF16, tag=f"tt{vc}", name=f"tt{vc}")
                      for vc in range(2)]
                for vc in range(2):
                    tv_ps = psTV.tile([128, QT], F32, tag="tv", name="tv")
                    for kc in range(8):
                        nc.tensor.matmul(tv_ps[:],
                                         g_aug[kc][:, vc * 128:(vc + 1) * 128],
                                         ex[kc // 2][:, (kc % 2) * QT:(kc % 2 + 1) * QT],
                                         start=(kc == 0), stop=(kc == 7))
                    nc.scalar.copy(tT[vc][:], tv_ps[:])

                # transpose es to q-major; reduce + reciprocal -> rec [128q, 4]
                esp = psO.tile([128, QT], BF16, tag="mm", name="mm")
                for c4 in range(4):
                    nc.tensor.transpose(esp[:, c4 * 128:(c4 + 1) * 128],
                                        es[:, c4 * 128:(c4 + 1) * 128],
                                        ident_b[:])
                sums = rc_p.tile([128, 4], F32, tag="sums", name="sums")
                nc.vector.tensor_reduce(
                    sums.rearrange("p (a c) -> p a c", c=1),
                    esp.rearrange("p (a c) -> p a c", a=4),
                    mybir.AxisListType.X, AO.add)
                rec = rc_p.tile([128, 4], F32, tag="rec", name="rec")
                nc.vector.reciprocal(rec[:], sums[:])

                prev = (tT, rec, xr, qt)

            emit_outproj(prev)

    nc.compile()
    return nc


def _get_nc():
    global _CACHED_NC
    if _CACHED_NC is None:
        _CACHED_NC = _build()
    return _CACHED_NC


def _run(inputs, trace=False, trace_kwargs=None):
    from ml_dtypes import bfloat16

    x = np.ascontiguousarray(np.asarray(inputs["x"], dtype=np.float32))
    wt = np.asarray(inputs["W_theta"], dtype=np.float32)
    wp = np.asarray(inputs["W_phi"], dtype=np.float32)
    wg = np.asarray(inputs["W_g"], dtype=np.float32)
    wo = np.asarray(inputs["W_o"], dtype=np.float32)
    gamma = np.float64(np.asarray(inputs["gamma"], dtype=np.float32))
    wo_s = (gamma * wo.astype(np.float64)).astype(np.float32)

    # host-side packing (not on the HW critical path)
    wtp = np.empty((128, 4, 128), dtype=np.float32)
    wgp = np.empty((128, 4, V), dtype=np.float32)
    wop = np.empty((128, 2, C), dtype=np.float32)
    for cc in range(4):
        wtp[:, cc, 0:64] = wt[cc * 128:(cc + 1) * 128, :]
        wtp[:, cc, 64:128] = wp[cc * 128:(cc + 1) * 128, :]
        wgp[:, cc, :] = wg[cc * 128:(cc + 1) * 128, :]
    for vc in range(2):
        wop[:, vc, :] = wo_s[vc * 128:(vc + 1) * 128, :]
    wtp_b = np.ascontiguousarray(wtp.reshape(128, 512).astype(bfloat16))
    wg_b = np.ascontiguousarray(wgp.reshape(128, 4 * V).astype(bfloat16))
    wo_b = np.ascontiguousarray(wop.reshape(128, 2 * C).astype(bfloat16))
    ident = np.eye(128, dtype=np.float32).astype(bfloat16)

    B = x.shape[0]
    assert B == N_CORES * IMG
    xs = x.reshape(B, S, C)
    xt_all = np.ascontiguousarray(
        xs.transpose(0, 2, 1).astype(bfloat16))        # [B, C, S] bf16
    xr_all = np.ascontiguousarray(xs.astype(bfloat16))  # residual, bf16
    in_maps = []
    for i in range(N_CORES):
        in_maps.append({
            "xt": xt_all[i * IMG:(i + 1) * IMG],
            "xr": xr_all[i * IMG:(i + 1) * IMG],
            "wtp": wtp_b, "wg": wg_b, "wo": wo_b,
            "ident": ident,
        })
    nc = _get_nc()
    kw = {}
    if trace:
        kw["trace"] = True
        if trace_kwargs:
            kw["trace_kwargs"] = trace_kwargs
    res = run_bass_kernel_spmd(nc, in_maps, core_ids=list(range(N_CORES)), **kw)
    outs = [np.asarray(res.results[i]["out"]).astype(np.float32).reshape(IMG, H, W, C) for i in range(N_CORES)]
    full = np.concatenate(outs, axis=0)
    return full, res


def kernel(**inputs):
    full, _ = _run(inputs, trace=False)
    return full


# revision 23
# speedup vs baseline: 1.0127x; 1.0127x over previous
"""SAGAN-style attention block on 8 TRN2 NeuronCores, data-parallel over batch.

Reference computation (per image, x: [64, 64, 512]):
    theta = x @ W_theta                     [4096, 64]
    phi   = maxpool2x2(x @ W_phi)           [1024, 64]
    g     = maxpool2x2(x @ W_g)             [1024, 256]
    beta  = softmax(theta @ phi.T, axis=-1) [4096, 1024]
    o     = (beta @ g) @ W_o                [4096, 512]
    out   = gamma * o + x
Sharding: batch 16 -> 2 images per core. No collectives.

Layout strategy (v3):
  - x is transposed and cast to bf16 on the HOST (xT [C, S]); no on-chip
    transpose phase. Residual x stays f32 via a separate input.
  - All matmuls run in bf16 (fast LDWEIGHTS, f32 PSUM accumulation).
  - Projections feature-major: thph [128, S] holds theta rows 0:64 and the
    phi pre-pool scratch rows 64:128 (one PSUM evacuation per q-tile);
    phiT [64, SK] (maxpool in the free dim), gT [256, SK] -> PE-transposed
    to key-major g_aug[kc] [128, 256].
  - scoresT [k, q] per (qt, kc) written pairwise into wide [128, 1024] PSUM
    tiles; ScalarE applies Exp once per pair on the PSUM->SBUF evacuation
    (no max subtraction: |scores| < 60, safe in f32).
  - Softmax denominators: pairwise-sum tree of the 4 wide exp tiles
    (DVE + GpSimd) -> es [128, 512]; PE-transpose es to q-major; DVE
    reduce + reciprocal gives rec [128q, 4]. No per-kc ones-matmuls.
  - attnV vc-outer: tmpT[v, q] accumulated over kc per vc so tT[0] is
    evacuated (ScalarE, bf16) while vc=1 still accumulates.
  - Output projection for q-tile N is emitted after the scores of q-tile
    N+1 (software pipelining) so the PE never waits on the tT evacuation.
  - out = (tmpT.T @ (gamma*W_o)) * rec + x fused on DVE, stored per-qs.
"""
import sys
import numpy as np

sys.path.insert(0, "/opt/trn_rl_repo")

from contextlib import ExitStack

import concourse.bass as bass
import concourse.tile as tile
from concourse import bacc, mybir
from concourse.bass_utils import run_bass_kernel_spmd

N_CORES = 8
IMG = 2            # images per core
H = W = 64
S = H * W          # 4096 queries per image
SK = S // 4        # 1024 keys after 2x2 maxpool
C = 512
D = C // 8         # 64
V = C // 2         # 256
QT = 512           # query tile
NQT = S // QT      # 8

F32 = mybir.dt.float32
BF16 = mybir.dt.bfloat16

_CACHED_NC = None


def _build():
    nc = bacc.Bacc("TRN2", target_bir_lowering=False, debug=False,
                   num_devices=N_CORES)
    xt_d = nc.dram_tensor("xt", [IMG, C, S], BF16, kind="ExternalInput").ap()
    xr_d = nc.dram_tensor("xr", [IMG, S, C], BF16, kind="ExternalInput").ap()
    wtp_d = nc.dram_tensor("wtp", [128, 4 * 128], BF16, kind="ExternalInput").ap()
    wg_d = nc.dram_tensor("wg", [128, 4 * V], BF16, kind="ExternalInput").ap()
    wo_d = nc.dram_tensor("wo", [128, 2 * C], BF16, kind="ExternalInput").ap()
    id_d = nc.dram_tensor("ident", [128, 128], BF16, kind="ExternalInput").ap()
    out_d = nc.dram_tensor("out", [IMG, S, C], BF16, kind="ExternalOutput").ap()

    AO = mybir.AluOpType
    dma_engines = [nc.sync, nc.scalar, nc.sync, nc.scalar]

    with tile.TileContext(nc) as tc, ExitStack() as ctx:
        # SBUF pools
        const_p = ctx.enter_context(tc.tile_pool(name="const", bufs=1))
        xt_p = ctx.enter_context(tc.tile_pool(name="xt", bufs=1))
        th_p = ctx.enter_context(tc.tile_pool(name="th", bufs=1))
        ga_p = ctx.enter_context(tc.tile_pool(name="ga", bufs=1))
        scr_p = ctx.enter_context(tc.tile_pool(name="scr", bufs=2))
        exp_p = ctx.enter_context(tc.tile_pool(name="exp", bufs=8))
        esw_p = ctx.enter_context(tc.tile_pool(name="esw", bufs=4))
        es_p = ctx.enter_context(tc.tile_pool(name="es", bufs=2))
        rc_p = ctx.enter_context(tc.tile_pool(name="rc", bufs=4))
        tt_p = ctx.enter_context(tc.tile_pool(name="tt", bufs=4))
        xr_p = ctx.enter_context(tc.tile_pool(name="xr", bufs=3))
        o_p = ctx.enter_context(tc.tile_pool(name="o", bufs=3))
        # PSUM pools (8 banks of 2KB/partition total)
        psA = ctx.enter_context(tc.tile_pool(name="psA", bufs=2, space="PSUM"))
        psTV = ctx.enter_context(tc.tile_pool(name="psTV", bufs=2, space="PSUM"))
        psO = ctx.enter_context(tc.tile_pool(name="psO", bufs=2, space="PSUM"))

        HS = S // 4

        def make_xt_tiles():
            return [[xt_p.tile([128, HS], BF16, tag=f"xT{cc}h{h}",
                               name=f"xT{cc}h{h}") for h in range(4)]
                    for cc in range(4)]

        def load_xt_wave(img, tiles, h):
            for cc in range(4):
                dma_engines[(h * 4 + cc) % 2].dma_start(
                    tiles[cc][h][:],
                    xt_d[img, cc * 128:(cc + 1) * 128, h * HS:(h + 1) * HS])

        # img0 first quarter-wave goes out before anything else (DMA-bound start)
        xTh0 = make_xt_tiles()
        load_xt_wave(0, xTh0, 0)

        # --- constants / weights (already packed + bf16 on host) ---
        wtp = const_p.tile([128, 4 * 128], BF16, tag="wtp", name="wtp")
        nc.sync.dma_start(wtp[:], wtp_d[:])
        ident_b = const_p.tile([128, 128], BF16, tag="ident", name="ident_b")
        nc.sync.dma_start(ident_b[:], id_d[:])
        wg = const_p.tile([128, 4 * V], BF16, tag="wg", name="wg")
        nc.sync.dma_start(wg[:], wg_d[:])
        wo = const_p.tile([128, 2 * C], BF16, tag="wo", name="wo")
        nc.sync.dma_start(wo[:], wo_d[:])

        for img in range(IMG):
            # ---------- load xT (host-transposed, bf16, quarter waves) ----------
            if img == 0:
                xTh = xTh0
                for h in range(1, 4):
                    load_xt_wave(0, xTh, h)
            else:
                xTh = make_xt_tiles()
                for h in range(4):
                    load_xt_wave(img, xTh, h)

            # ---------- projections (feature-major) ----------
            # thph rows 0:64 = thetaT [64, S]; rows 64:128 = phi pre-pool scratch
            thph = th_p.tile([128, S], BF16, tag="thph", name="thph")
            thetaT = thph[0:64, :]
            phiT = th_p.tile([64, SK], BF16, tag="phiT", name="phiT")
            gTp = [th_p.tile([128, SK], BF16, tag=f"gTp{vc}", name=f"gTp{vc}")
                   for vc in range(2)]

            for qt in range(NQT):
                qsl = slice(qt * QT, (qt + 1) * QT)
                hh = qt // 2
                hsl = slice((qt % 2) * QT, (qt % 2 + 1) * QT)
                tp_ps = psA.tile([128, 2 * QT], F32, tag="mm", name="mm")
                for cc in range(4):
                    nc.tensor.matmul(tp_ps[:, 0:QT],
                                     wtp[:, cc * 128:(cc + 1) * 128],
                                     xTh[cc][hh][:, hsl], start=(cc == 0), stop=(cc == 3))
                # one evacuation for theta+phi
                nc.vector.tensor_copy(thph[:, qsl], tp_ps[:, 0:QT])
                # phi: maxpool 2x2 in the free dim:
                # QT=512 covers 8 rows of w=64 -> [p, a=4, b=2, c=32, d=2]
                pv = thph[64:128, qsl].rearrange(
                    "p (a b c d) -> p a b c d", b=2, c=32, d=2)
                m1 = scr_p.tile([64, 256], BF16, tag="m1", name="m1")
                ma = m1[:, :128].rearrange("p (a c) -> p a c", a=4)
                mb = m1[:, 128:].rearrange("p (a c) -> p a c", a=4)
                po = phiT[:, qt * 128:(qt + 1) * 128].rearrange(
                    "p (a c) -> p a c", a=4)
                nc.vector.tensor_tensor(ma, pv[:, :, 0, :, 0], pv[:, :, 0, :, 1],
                                        AO.max)
                nc.vector.tensor_tensor(mb, pv[:, :, 1, :, 0], pv[:, :, 1, :, 1],
                                        AO.max)
                nc.vector.tensor_tensor(po, ma, mb, AO.max)

                for vc in range(2):
                    g_ps = psO.tile([128, QT], F32, tag="mm", name="mm")
                    for cc in range(4):
                        nc.tensor.matmul(
                            g_ps[:], wg[:, cc * V + vc * 128: cc * V + (vc + 1) * 128],
                            xTh[cc][hh][:, hsl], start=(cc == 0), stop=(cc == 3))
                    g_sb = scr_p.tile([128, QT], BF16, tag="g_sb", name="g_sb")
                    nc.scalar.copy(g_sb[:], g_ps[:])
                    gv = g_sb.rearrange("p (a b c d) -> p a b c d", b=2, c=32, d=2)
                    m2 = scr_p.tile([128, 256], BF16, tag="m2", name="m2")
                    ga = m2[:, :128].rearrange("p (a c) -> p a c", a=4)
                    gb = m2[:, 128:].rearrange("p (a c) -> p a c", a=4)
                    go = gTp[vc][:, qt * 128:(qt + 1) * 128].rearrange(
                        "p (a c) -> p a c", a=4)
                    nc.vector.tensor_tensor(ga, gv[:, :, 0, :, 0], gv[:, :, 0, :, 1],
                                            AO.max)
                    nc.vector.tensor_tensor(gb, gv[:, :, 1, :, 0], gv[:, :, 1, :, 1],
                                            AO.max)
                    nc.vector.tensor_tensor(go, ga, gb, AO.max)

            # g -> key-major: g_aug[kc] = [128k, 256v] (bf16 PE transpose)
            g_aug = []
            for kc in range(8):
                ga_t = ga_p.tile([128, V], BF16, tag=f"gaug{kc}", name=f"gaug{kc}")
                tp = psO.tile([128, 512], BF16, tag="mm", name="mm")
                for vc in range(2):
                    nc.tensor.transpose(tp[:, vc * 128:(vc + 1) * 128],
                                        gTp[vc][:, kc * 128:(kc + 1) * 128],
                                        ident_b[:])
                nc.vector.tensor_copy(ga_t[:], tp[:, 0:V])
                g_aug.append(ga_t)

            # ---------- attention, software-pipelined over q-tiles ----------
            prev = None  # (tT, rec, xr, qt) of the previous q-tile

            def emit_outproj(state):
                tT, rec, xr, qt = state
                for qs in range(4):
                    ssl = slice(qs * 128, (qs + 1) * 128)
                    o_ps = psO.tile([128, C], F32, tag="mm", name="mm")
                    for vc in range(2):
                        nc.tensor.matmul(o_ps[:], tT[vc][:, ssl],
                                         wo[:, vc * C:(vc + 1) * C],
                                         start=(vc == 0), stop=(vc == 1))
                    ot = o_p.tile([128, C], BF16, tag="ot", name="ot")
                    # out = (o * (1/sum)) + x   (gamma pre-folded into W_o)
                    nc.vector.scalar_tensor_tensor(
                        ot[:], o_ps[:], rec[:, qs:qs + 1],
                        xr[:, qs * C:(qs + 1) * C], AO.mult, AO.add)
                    q0 = qt * QT + qs * 128
                    nc.sync.dma_start(out_d[img, q0:q0 + 128, :], ot[:])

            for qt in range(NQT):
                qsl = slice(qt * QT, (qt + 1) * QT)
                # residual x for this q-tile (consumed next iteration)
                xr = xr_p.tile([128, 4 * C], BF16, tag="xr", name="xr")
                nc.sync.dma_start(
                    xr.rearrange("p (a c) -> p a c", a=4),
                    xr_d[img, qt * QT:(qt + 1) * QT, :].rearrange(
                        "(a p) c -> p a c", p=128))

                # scoresT [k, q] pairwise into wide PSUM; one Exp per pair
                ex = []
                for kp in range(4):
                    sc_ps = psA.tile([128, 2 * QT], F32, tag="mm", name="mm")
                    for h in range(2):
                        kc = 2 * kp + h
                        nc.tensor.matmul(sc_ps[:, h * QT:(h + 1) * QT],
                                         phiT[:, kc * 128:(kc + 1) * 128],
                                         thetaT[:, qsl], start=True, stop=True)
                    e_t = exp_p.tile([128, 2 * QT], BF16, tag="exp", name="exp")
                    nc.scalar.activation(e_t[:], sc_ps[:],
                                         mybir.ActivationFunctionType.Exp)
                    ex.append(e_t)

                # output projection of the PREVIOUS q-tile (pipelined):
                # PE fills the wait on exp with useful work, DVE runs the
                # STTs while the scores stream
                if prev is not None:
                    emit_outproj(prev)

                # denominator pre-sum tree (DVE)
                w01 = esw_p.tile([128, 2 * QT], BF16, tag="esw", name="esw")
                w23 = esw_p.tile([128, 2 * QT], BF16, tag="esw", name="esw")
                nc.vector.tensor_tensor(w01[:], ex[0][:], ex[1][:], AO.add)
                nc.vector.tensor_tensor(w23[:], ex[2][:], ex[3][:], AO.add)
                wsum = esw_p.tile([128, 2 * QT], BF16, tag="esw", name="esw")
                nc.vector.tensor_tensor(wsum[:], w01[:], w23[:], AO.add)
                es = es_p.tile([128, QT], BF16, tag="es", name="es")
                nc.vector.tensor_tensor(es[:], wsum[:, 0:QT], wsum[:, QT:2 * QT],
                                        AO.add)

                # attnV vc-outer: tT[vc] evacuates while the other vc runs
                tT = [tt_p.tile([128, QT], BF16, tag=f"tt{vc}", name=f"tt{vc}")
                      for vc in range(2)]
                for vc in range(2):
                    tv_ps = psTV.tile([128, QT], F32, tag="tv", name="tv")
                    for kc in range(8):
                        nc.tensor.matmul(tv_ps[:],
                                         g_aug[kc][:, vc * 128:(vc + 1) * 128],
                                         ex[kc // 2][:, (kc % 2) * QT:(kc % 2 + 1) * QT],
                                         start=(kc == 0), stop=(kc == 7))
                    nc.scalar.copy(tT[vc][:], tv_ps[:])

                # transpose es to q-major; reduce + reciprocal -> rec [128q, 4]
                esp = psO.tile([128, QT], BF16, tag="mm", name="mm")
                for c4 in range(4):
                    nc.tensor.transpose(esp[:, c4 * 128:(c4 + 1) * 128],
                                        es[:, c4 * 128:(c4 + 1) * 128],
                                        ident_b[:])
                sums = rc_p.tile([128, 4], F32, tag="sums", name="sums")
                nc.vector.tensor_reduce(
                    sums.rearrange("p (a c) -> p a c", c=1),
                    esp.rearrange("p (a c) -> p a c", a=4),
                    mybir.AxisListType.X, AO.add)
                rec = rc_p.tile([128, 4], F32, tag="rec", name="rec")
                nc.vector.reciprocal(rec[:], sums[:])

                prev = (tT, rec, xr, qt)

            emit_outproj(prev)

    nc.compile()
    return nc


def _get_nc():
    global _CACHED_NC
    if _CACHED_NC is None:
        _CACHED_NC = _build()
    return _CACHED_NC


def _run(inputs, trace=False, trace_kwargs=None):
    from ml_dtypes import bfloat16

    x = np.ascontiguousarray(np.asarray(inputs["x"], dtype=np.float32))
    wt = np.asarray(inputs["W_theta"], dtype=np.float32)
    wp = np.asarray(inputs["W_phi"], dtype=np.float32)
    wg = np.asarray(inputs["W_g"], dtype=np.float32)
    wo = np.asarray(inputs["W_o"], dtype=np.float32)
    gamma = np.float64(np.asarray(inputs["gamma"], dtype=np.float32))
    wo_s = (gamma * wo.astype(np.float64)).astype(np.float32)

    # host-side packing (not on the HW critical path)
    wtp = np.empty((128, 4, 128), dtype=np.float32)
    wgp = np.empty((128, 4, V), dtype=np.float32)
    wop = np.empty((128, 2, C), dtype=np.float32)
    for cc in range(4):
        wtp[:, cc, 0:64] = wt[cc * 128:(cc + 1) * 128, :]
        wtp[:, cc, 64:128] = wp[cc * 128:(cc + 1) * 128, :]
        wgp[:, cc, :] = wg[cc * 128:(cc + 1) * 128, :]
    for vc in range(2):
        wop[:, vc, :] = wo_s[vc * 128:(vc + 1) * 128, :]
    wtp_b = np.ascontiguousarray(wtp.reshape(128, 512).astype(bfloat16))
    wg_b = np.ascontiguousarray(wgp.reshape(128, 4 * V).astype(bfloat16))
    wo_b = np.ascontiguousarray(wop.reshape(128, 2 * C).astype(bfloat16))
    ident = np.eye(128, dtype=np.float32).astype(bfloat16)

    B = x.shape[0]
    assert B == N_CORES * IMG
    xs = x.reshape(B, S, C)
    xt_all = np.ascontiguousarray(
        xs.transpose(0, 2, 1).astype(bfloat16))        # [B, C, S] bf16
    xr_all = np.ascontiguousarray(xs.astype(bfloat16))  # residual, bf16
    in_maps = []
    for i in range(N_CORES):
        in_maps.append({
            "xt": xt_all[i * IMG:(i + 1) * IMG],
            "xr": xr_all[i * IMG:(i + 1) * IMG],
            "wtp": wtp_b, "wg": wg_b, "wo": wo_b,
            "ident": ident,
        })
    nc = _get_nc()
    kw = {}
    if trace:
        kw["trace"] = True
        if trace_kwargs:
            kw["trace_kwargs"] = trace_kwargs
    res = run_bass_kernel_spmd(nc, in_maps, core_ids=list(range(N_CORES)), **kw)
    outs = [np.asarray(res.results[i]["out"]).astype(np.float32).reshape(IMG, H, W, C) for i in range(N_CORES)]
    full = np.concatenate(outs, axis=0)
    return full, res


def kernel(**inputs):
    full, _ = _run(inputs, trace=False)
    return full
